# revision 44
# baseline (speedup 1.0000x reference)
"""Trainium2 Bass kernel for the audio/visual contrastive loss.

Strategy: K-parallel sharding of the visual matmul in fp8.

- Host casts inputs to fp8-e4m3 (W_v pre-scaled x256, W_a x32 -- any
  per-matrix scale cancels in the L2 normalization) and pre-transposes the
  activations to k-major, so the device does no PE transposes.
- Each core contracts a 8640-wide K slice of the visual matmul with
  fp8 DoubleRow matmuls (two 128-deep k-planes per instruction).
- The audio embedding (K=1280, tiny) is computed fully on every core
  after the visual stream, so the cross-core reduction only carries the
  visual partial E.T and audio stays off the critical path.
- Reduction: bf16 ReduceScatter (each core gets a 64-sample chunk of the
  reduced visual E.T), local L2-normalize of that chunk (scaled x16 for
  fp8 range), then an fp8 AllGather of the normalized embeddings.
- Tail (redundant on every core): fp8 DoubleRow Gram blocks, exp with
  row-accumulate for the denominator, diagonal extraction via
  identity-mask + row-reduce for the numerator, log/mean in column space.
"""

import sys

sys.path.insert(0, "/opt/trn_rl_repo")

import ml_dtypes
import numpy as np

import concourse.bass as bass
import concourse.mybir as mybir
import concourse.tile as tile
from concourse import bacc, bass_utils
from concourse.bass import ts
from concourse.masks import make_identity

N_CORES = 8
B = 256          # batch
S = 2 * B        # samples per modality after the pair-concat
D = 512          # embedding dim
KV_TOT = 3 * 5 * 48 * 96   # 69120 visual features (lower half)
KV = KV_TOT // N_CORES     # 8640 per core
KP = 8704                  # padded to 34 * 256
NKT = KP // 256            # 34 double-k-tiles
KA = 1280                  # audio features (not sharded)
NKA = KA // 256            # 5 double-k-tiles
F32 = mybir.dt.float32
F32R = mybir.dt.float32r
BF16 = mybir.dt.bfloat16
F8 = mybir.dt.float8e4
NP_F8 = ml_dtypes.float8_e4m3
AF = mybir.ActivationFunctionType
DR = mybir.MatmulPerfMode.DoubleRow

SC_V = 256.0    # host scale on W_v so fp8 sees ~unit-variance values
SC_A = 32.0     # host scale on W_a
EMB_SC = 16.0   # scale on normalized embeddings for fp8; Gram gets x256
GRAM_RCP = 1.0 / (EMB_SC * EMB_SC)   # exp(scale * raw_gram)

_CACHE = {}


def build():
    nc = bacc.Bacc("TRN2", target_bir_lowering=False, debug=False,
                   num_devices=N_CORES)

    # k-major fp8 inputs, pre-packed on host for DoubleRow + big DMAs
    xv_d = nc.dram_tensor("xv", [NKT, 128, 2, S], F8, kind="ExternalInput")
    wv_d = nc.dram_tensor("wv", [NKT, 128, 2, D], F8, kind="ExternalInput")
    xa_d = nc.dram_tensor("xa", [NKA, 128, 2, S], F8, kind="ExternalInput")
    wa_d = nc.dram_tensor("wa", [NKA, 128, 2, D], F8, kind="ExternalInput")
    loss_d = nc.dram_tensor("loss", [1, 1], F32, kind="ExternalOutput")

    with tile.TileContext(nc) as tc:
        with tc.tile_pool(name="const", bufs=1) as constp, \
             tc.tile_pool(name="inp", bufs=1) as inp, \
             tc.tile_pool(name="emb", bufs=1) as embp, \
             tc.tile_pool(name="dram", bufs=1, space="DRAM") as dramp, \
             tc.tile_pool(name="pbig", bufs=1, space="PSUM") as pbig, \
             tc.tile_pool(name="psmall", bufs=1, space="PSUM") as psmall:
            ident = constp.tile([128, 128], F32)
            make_identity(nc, ident[:])
            ident4 = constp.tile([128, S], F32)
            for q in range(4):
                nc.vector.tensor_copy(ident4[:, ts(q, 128)], ident[:])
            ones_f = constp.tile([128, 1], F32)
            nc.vector.memset(ones_f[:], 1.0)
            ones_r = constp.tile([128, 1], F32R)
            nc.vector.tensor_copy(ones_r[:], ones_f[:])
            ones_row_f = constp.tile([1, 128], F32)
            nc.vector.memset(ones_row_f[:], 1.0)
            ones_row_r = constp.tile([1, 128], F32R)
            nc.vector.tensor_copy(ones_row_r[:], ones_row_f[:])
            neginv_f = constp.tile([128, 1], F32)
            nc.vector.memset(neginv_f[:], -1.0 / B)
            neginv_r = constp.tile([128, 1], F32R)
            nc.vector.tensor_copy(neginv_r[:], neginv_f[:])
            ones_f8 = constp.tile([128, 1], F8)
            nc.vector.tensor_copy(ones_f8[:], ones_f[:])
            warm = constp.tile([1, 4], F32)
            nc.vector.memset(warm[:], 1.0)
            for fn in (AF.Exp, AF.Sqrt, AF.Ln):
                nc.scalar.activation(warm[:], warm[:], fn)

            # ---- visual input stream: interleaved x/w chunks ----
            xv_sb = inp.tile([128, NKT, 2, S], F8)
            wv_sb = inp.tile([128, NKT, 2, D], F8)
            sizes = [10, 8, 6, 4, 3, 2, 1]
            bounds, k0 = [], 0
            for sz in sizes:
                bounds.append((k0, k0 + sz))
                k0 += sz
            for k0, k1 in bounds:
                nc.sync.dma_start(
                    out=xv_sb[:, k0:k1],
                    in_=xv_d.ap()[k0:k1].rearrange("kt p pl c -> p kt pl c"))
                nc.sync.dma_start(
                    out=wv_sb[:, k0:k1],
                    in_=wv_d.ap()[k0:k1].rearrange("kt p pl c -> p kt pl c"))

            # ---- visual partial E.T, k-outer so PE chases the stream ----
            psum_v = [pbig.tile([128, S], F32, tag=f"pa{d}", name=f"psum_v{d}")
                      for d in range(4)]
            for kt in range(NKT):
                for d in range(4):
                    for h in range(2):
                        nc.tensor.matmul(
                            psum_v[d][:, ts(h, 256)],
                            wv_sb[:, kt, :, ts(d, 128)],
                            xv_sb[:, kt, :, ts(h, 256)],
                            start=(kt == 0 and h == 0), stop=(kt == NKT - 1),
                            perf_mode=DR, skip_group_check=True)

            rs_in = dramp.tile([8, 128, 4, 64], BF16)
            e_sb = [embp.tile([128, S], BF16, name=f"e_sb{d}",
                              tag=f"esb{d}")
                    for d in range(4)]
            for d in range(4):
                if d % 2 == 0:
                    nc.vector.tensor_copy(e_sb[d][:], psum_v[d][:])
                else:
                    nc.scalar.copy(e_sb[d][:], psum_v[d][:])
                nc.sync.dma_start(
                    out=rs_in[:, :, d].rearrange("j p c -> p j c"),
                    in_=e_sb[d].rearrange("p (j c) -> p j c", j=8))

            # ---- ReduceScatter: core j gets reduced E.T for samples
            # [64j, 64j+64) as [4, 128, 64] (d-tile, partition, col) ----
            rs_out = dramp.tile([128, 4, 64], BF16)
            nc.gpsimd.collective_compute(
                "ReduceScatter", mybir.AluOpType.add,
                replica_groups=[list(range(N_CORES))],
                ins=[rs_in.opt()], outs=[rs_out.opt()],
            )

            # ---- audio (off the critical path: full K on every core) ----
            # share slots with e_sb so the audio input DMAs queue AFTER the
            # RS payload writes on the DMA engines (WAR on the freed buffers)
            xa_sb = embp.tile([128, NKA, 2, S], F8, tag="esb0", name="xa_sb")
            nc.sync.dma_start(
                out=xa_sb[:],
                in_=xa_d.ap().rearrange("kt p pl c -> p kt pl c"))
            wa_sb = embp.tile([128, NKA, 2, D], F8, tag="esb1", name="wa_sb")
            nc.sync.dma_start(
                out=wa_sb[:],
                in_=wa_d.ap().rearrange("kt p pl c -> p kt pl c"))
            psum_a = [pbig.tile([128, S], F32, tag=f"pa{d}", name=f"psum_a{d}")
                      for d in range(4)]
            for d in range(4):
                for kt in range(NKA):
                    for h in range(2):
                        nc.tensor.matmul(
                            psum_a[d][:, ts(h, 256)],
                            wa_sb[:, kt, :, ts(d, 128)],
                            xa_sb[:, kt, :, ts(h, 256)],
                            start=(kt == 0 and h == 0), stop=(kt == NKA - 1),
                            perf_mode=DR, skip_group_check=True)
            # audio norms: colsum of squares -> 16/sqrt -> broadcast -> scale
            ax = embp.tile([128, 4, S], F32R)
            for d in range(4):
                nc.vector.tensor_copy(ax[:, d, :], psum_a[d][:])
            sq_a = embp.tile([128, 4, S], F32R)
            for d in range(4):
                nc.vector.tensor_mul(sq_a[:, d, :], ax[:, d, :], ax[:, d, :])
            ps_na = psmall.tile([1, S], F32, tag="rowp", name="ps_na")
            for d in range(4):
                nc.tensor.matmul(ps_na[:], ones_r[:], sq_a[:, d, :],
                                 start=(d == 0), stop=(d == 3))
            sn_a = embp.tile([1, S], F32)
            nc.scalar.activation(sn_a[:], ps_na[:], AF.Sqrt,
                                 scale=float(1.0 / (EMB_SC * EMB_SC)))
            rn_a_f = embp.tile([1, S], F32)
            nc.vector.reciprocal(rn_a_f[:], sn_a[:])
            rn_a = embp.tile([1, S], F32R)
            nc.vector.tensor_copy(rn_a[:], rn_a_f[:])
            ps_bca = psmall.tile([128, S], F32, tag="bcp", name="ps_bca")
            nc.tensor.matmul(ps_bca[:], ones_row_r[:], rn_a[:],
                             start=True, stop=True)
            bc_a = embp.tile([128, S], F32)
            nc.vector.tensor_copy(bc_a[:], ps_bca[:])
            # ea[tt][p, pl, s] = normalized audio emb, d = 128*(2tt+pl)+p
            ea = [embp.tile([128, 2, S], F8, name=f"ea{t}") for t in range(2)]
            for d in range(4):
                nc.vector.tensor_mul(ea[d // 2][:, d % 2, :],
                                     ax[:, d, :], bc_a[:])

            # ---- normalize my reduced 64-sample chunk, AllGather fp8 ----
            red = embp.tile([128, 4, 64], BF16)
            nc.sync.dma_start(out=red[:], in_=rs_out[:])
            sq_v = embp.tile([128, 256], F32R)
            nc.vector.tensor_mul(sq_v[:], red.rearrange("p t c -> p (t c)"),
                                 red.rearrange("p t c -> p (t c)"))
            ps_nv = psmall.tile([1, 64], F32, tag="rowp", name="ps_nv")
            for t in range(4):
                nc.tensor.matmul(ps_nv[:], ones_r[:], sq_v[:, ts(t, 64)],
                                 start=(t == 0), stop=(t == 3))
            sn_v = embp.tile([1, 64], F32)
            nc.scalar.activation(sn_v[:], ps_nv[:], AF.Sqrt,
                                 scale=float(1.0 / (EMB_SC * EMB_SC)))
            rn_v_f = embp.tile([1, 64], F32)
            nc.vector.reciprocal(rn_v_f[:], sn_v[:])
            bc_v = embp.tile([128, 64], F32)
            nc.gpsimd.partition_broadcast(bc_v[:], rn_v_f[:])
            ag_sb = embp.tile([128, 4, 64], F8)
            nc.vector.tensor_mul(ag_sb[:], red[:],
                                 bc_v.rearrange("p (o c) -> p o c", o=1)
                                 .broadcast_to([128, 4, 64]))

            # chunk layout is partition-major so the gathered embeddings can
            # be pulled back in one DMA
            ag_in = dramp.tile([128, 4, 64], F8)
            nc.sync.dma_start(out=ag_in[:], in_=ag_sb[:])
            ag_out = dramp.tile([8, 128, 4, 64], F8)
            nc.gpsimd.collective_compute(
                "AllGather", mybir.AluOpType.bypass,
                replica_groups=[list(range(N_CORES))],
                ins=[ag_in.opt()], outs=[ag_out.opt()],
            )
            # ev[p, j, t, c]: normalized visual emb,
            # d = 128*t+p, sample = 64j+c
            ev = embp.tile([128, 8, 4, 64], F8)
            nc.sync.dma_start(
                out=ev[:], in_=ag_out.rearrange("j p t c -> p j t c"))

            # ---------------- tail: Gram, exp, loss ----------------
            with tc.tile_pool(name="tail", bufs=1) as tp:
                # av Gram: psum_av[m][i, j] = a_{128m+i} . v_j  (x256)
                psum_av = [pbig.tile([128, S], F32, tag=f"pa{m}",
                                     name=f"psum_av{m}") for m in range(4)]
                psum_q = psmall.tile([128, S], F32, tag="bcp", name="psum_q")
                for m in range(4):
                    for tt in range(2):
                        for h in range(2):
                            nc.tensor.matmul(
                                psum_av[m][:, ts(h, 256)],
                                ea[tt][:, :, ts(m, 128)],
                                ev[:, 4 * h:4 * h + 4, 2 * tt:2 * tt + 2]
                                    .rearrange("p j pl c -> p pl j c"),
                                start=(tt == 0 and h == 0), stop=(tt == 1),
                                perf_mode=DR, skip_group_check=True)
                # quadrants: [a1a2 m0, a1a2 m1, v1v2 m0, v1v2 m1]
                for q in range(4):
                    m = q % 2
                    for tt in range(2):
                        if q < 2:    # a1 block m  x  a2 block m
                            lhsT = ea[tt][:, :, ts(m, 128)]
                            rhs = ea[tt][:, :, 256 + 128 * m:384 + 128 * m]
                        else:        # v1 block m  x  v2 block m
                            lhsT = ev[:, 2 * m:2 * m + 2, 2 * tt:2 * tt + 2] \
                                .rearrange("p j pl c -> p pl j c")
                            rhs = ev[:, 4 + 2 * m:6 + 2 * m, 2 * tt:2 * tt + 2] \
                                .rearrange("p j pl c -> p pl j c")
                        nc.tensor.matmul(
                            psum_q[:, ts(q, 128)], lhsT, rhs,
                            start=(q == 0 and tt == 0), stop=(tt == 1),
                            perf_mode=DR, skip_group_check=True)

                # numerator first: raw diagonals straight off the PSUM
                # Grams (no need to wait for the big exps). av block m holds
                # (a?, v1) diag at cols 128j and (a?, v2) at 256+128j, j=m%2.
                mk = tp.tile([128, 4, 256], F32, tag="mk", name="mk")
                qd = tp.tile([128, 12], F32)
                for m in range(4):
                    j = m % 2
                    part = psum_av[m] \
                        .rearrange("p (g c) -> p g c", g=4)[:, j:j + 3:2, :]
                    nc.vector.tensor_mul(
                        mk[:, m].rearrange("p (g c) -> p g c", g=2),
                        part, ident4.rearrange("p (g c) -> p g c",
                                               g=4)[:, 0:2, :])
                    for gi in range(2):
                        col = 6 * (m % 2) + 2 * (m // 2) + gi
                        nc.vector.reduce_sum(
                            qd[:, col:col + 1], mk[:, m, ts(gi, 128)],
                            axis=mybir.AxisListType.X)
                # raw diagonals of (a1,a2) and (v1,v2) quadrants
                mq = tp.tile([128, S], F32, tag="mq", name="mq")
                nc.vector.tensor_mul(mq[:], psum_q[:], ident4[:])
                for q in range(4):
                    col = 6 * (q % 2) + 4 + (q // 2)
                    nc.vector.reduce_sum(qd[:, col:col + 1], mq[:, ts(q, 128)],
                                         axis=mybir.AxisListType.X)

                # denominator: rowsum of exp(G/256) over all 512 visual;
                # the exp'd matrix itself is not needed, only the accum
                junk = tp.tile([128, S], F32, tag="junk", name="junk")
                den4 = tp.tile([128, 4], F32)
                for m in range(4):
                    nc.scalar.activation(junk[:], psum_av[m][:],
                                         AF.Exp, scale=float(GRAM_RCP),
                                         accum_out=den4[:, m:m + 1])
                eqd = tp.tile([128, 12], F32)
                nc.scalar.activation(eqd[:], qd[:], AF.Exp,
                                     scale=float(GRAM_RCP))

                # nd[:, 0:2] = numerator, nd[:, 2:4] = denominator (cols =
                # batch halves)
                nd = tp.tile([128, 4], F32)
                for j in range(2):
                    nc.vector.tensor_add(nd[:, 2 + j:3 + j], den4[:, j:j + 1],
                                         den4[:, j + 2:j + 3])
                # cols [6j, 6j+6) hold all six exp'd numerator terms for
                # batch half j
                for j in range(2):
                    nc.vector.reduce_sum(nd[:, j:j + 1],
                                         eqd[:, 6 * j:6 * j + 6],
                                         axis=mybir.AxisListType.X)

                # loss = -mean(log num - log den); the -1/B mean factor is
                # folded into the summing matmul's stationary vector
                lg = tp.tile([128, 4], F32R)
                nc.scalar.activation(lg[:], nd[:], AF.Ln)
                dif = tp.tile([128, 2], F32R)
                nc.vector.tensor_sub(dif[:], lg[:, 0:2], lg[:, 2:4])
                ps_l = psmall.tile([1, 2], F32, tag="rowp", name="ps_l")
                nc.tensor.matmul(ps_l[:], neginv_r[:], dif[:],
                                 start=True, stop=True)
                loss_sb = tp.tile([1, 1], F32)
                nc.vector.reduce_sum(loss_sb[:], ps_l[:],
                                     axis=mybir.AxisListType.X)
                nc.sync.dma_start(out=loss_d.ap(), in_=loss_sb[:])

    nc.compile()
    return nc


def _get_nc():
    if "nc" not in _CACHE:
        _CACHE["nc"] = build()
    return _CACHE["nc"]


def _pack_kmajor(m, nkt, width):
    """[K, width] k-major -> [nkt, 128, 2, width] fp8 DoubleRow layout."""
    out = m.reshape(nkt, 2, 128, width).transpose(0, 2, 1, 3)
    return np.ascontiguousarray(out).astype(NP_F8)


def _shard_inputs(a_1, v_1, a_2, v_2, W_a, W_v):
    # audio: (2b,1,80,16) -> (512, 1280); replicated on every core
    A = np.concatenate([a_1, a_2], axis=0).reshape(S, KA)
    xa = _pack_kmajor(np.ascontiguousarray(A.T), NKA, S)
    wa = _pack_kmajor(W_a * np.float32(SC_A), NKA, D)
    # visual: lower half, flattened in (c,t,r,w) order; W_v rows permuted
    # from the reference's (t,c) order to match.
    V = np.concatenate([v_1, v_2], axis=0)
    V = V.reshape(S, 15, 96, 96)[:, :, 48:, :].reshape(S, KV_TOT)
    Vt = np.ascontiguousarray(V.T)
    Wvp = np.ascontiguousarray(
        W_v.reshape(5, 3, 48 * 96, D).transpose(1, 0, 2, 3)
    ).reshape(KV_TOT, D) * np.float32(SC_V)

    in_maps = []
    for c in range(N_CORES):
        vt_p = np.zeros((KP, S), np.float32)
        vt_p[:KV] = Vt[c * KV:(c + 1) * KV]
        xv = _pack_kmajor(vt_p, NKT, S)
        wv_p = np.zeros((KP, D), np.float32)
        wv_p[:KV] = Wvp[c * KV:(c + 1) * KV]
        wv = _pack_kmajor(wv_p, NKT, D)
        in_maps.append({"xv": xv, "wv": wv, "xa": xa, "wa": wa})
    return in_maps


def kernel(a_1, v_1, a_2, v_2, W_a, W_v):
    nc = _get_nc()
    in_maps = _shard_inputs(np.asarray(a_1, np.float32),
                            np.asarray(v_1, np.float32),
                            np.asarray(a_2, np.float32),
                            np.asarray(v_2, np.float32),
                            np.asarray(W_a, np.float32),
                            np.asarray(W_v, np.float32))
    res = bass_utils.run_bass_kernel_spmd(nc, in_maps,
                                          core_ids=list(range(N_CORES)))
    return np.asarray(res.results[0]["loss"], np.float32).reshape(())


# revision 45
# speedup vs baseline: 1.0093x; 1.0093x over previous
"""Trainium2 Bass kernel for the audio/visual contrastive loss.

Strategy: K-parallel sharding of the visual matmul in fp8.

- Host casts inputs to fp8-e4m3 (W_v pre-scaled x256, W_a x32 -- any
  per-matrix scale cancels in the L2 normalization) and pre-transposes the
  activations to k-major, so the device does no PE transposes.
- Each core contracts a 8640-wide K slice of the visual matmul with
  fp8 DoubleRow matmuls (two 128-deep k-planes per instruction).
- The audio embedding (K=1280, tiny) is computed fully on every core
  after the visual stream, so the cross-core reduction only carries the
  visual partial E.T and audio stays off the critical path.
- Reduction: bf16 ReduceScatter (each core gets a 64-sample chunk of the
  reduced visual E.T), local L2-normalize of that chunk (scaled x16 for
  fp8 range), then an fp8 AllGather of the normalized embeddings.
- Tail (redundant on every core): fp8 DoubleRow Gram blocks, exp with
  row-accumulate for the denominator, diagonal extraction via
  identity-mask + row-reduce for the numerator, log/mean in column space.
"""

import sys

sys.path.insert(0, "/opt/trn_rl_repo")

import ml_dtypes
import numpy as np

import concourse.bass as bass
import concourse.mybir as mybir
import concourse.tile as tile
from concourse import bacc, bass_utils
from concourse.bass import ts
from concourse.masks import make_identity

N_CORES = 8
B = 256          # batch
S = 2 * B        # samples per modality after the pair-concat
D = 512          # embedding dim
KV_TOT = 3 * 5 * 48 * 96   # 69120 visual features (lower half)
KV = KV_TOT // N_CORES     # 8640 per core
KP = 8704                  # padded to 34 * 256
NKT = KP // 256            # 34 double-k-tiles
KA = 1280                  # audio features (not sharded)
NKA = KA // 256            # 5 double-k-tiles
F32 = mybir.dt.float32
F32R = mybir.dt.float32r
BF16 = mybir.dt.bfloat16
F8 = mybir.dt.float8e4
NP_F8 = ml_dtypes.float8_e4m3
AF = mybir.ActivationFunctionType
DR = mybir.MatmulPerfMode.DoubleRow

SC_V = 256.0    # host scale on W_v so fp8 sees ~unit-variance values
SC_A = 32.0     # host scale on W_a
EMB_SC = 16.0   # scale on normalized embeddings for fp8; Gram gets x256
GRAM_RCP = 1.0 / (EMB_SC * EMB_SC)   # exp(scale * raw_gram)

_CACHE = {}


def build():
    nc = bacc.Bacc("TRN2", target_bir_lowering=False, debug=False,
                   num_devices=N_CORES)

    # k-major fp8 inputs, pre-packed on host for DoubleRow + big DMAs
    xv_d = nc.dram_tensor("xv", [NKT, 128, 2, S], F8, kind="ExternalInput")
    wv_d = nc.dram_tensor("wv", [NKT, 128, 2, D], F8, kind="ExternalInput")
    au_d = nc.dram_tensor("au", [NKA, 128, 2, 2 * D], F8,
                          kind="ExternalInput")
    loss_d = nc.dram_tensor("loss", [1, 1], F32, kind="ExternalOutput")

    with tile.TileContext(nc) as tc:
        with tc.tile_pool(name="const", bufs=1) as constp, \
             tc.tile_pool(name="inp", bufs=1) as inp, \
             tc.tile_pool(name="emb", bufs=1) as embp, \
             tc.tile_pool(name="dram", bufs=1, space="DRAM") as dramp, \
             tc.tile_pool(name="pbig", bufs=1, space="PSUM") as pbig, \
             tc.tile_pool(name="psmall", bufs=1, space="PSUM") as psmall:
            ident = constp.tile([128, 128], F32)
            make_identity(nc, ident[:])
            ident4 = constp.tile([128, S], F32)
            for q in range(4):
                nc.vector.tensor_copy(ident4[:, ts(q, 128)], ident[:])
            ones_f = constp.tile([128, 1], F32)
            nc.vector.memset(ones_f[:], 1.0)
            ones_r = constp.tile([128, 1], F32R)
            nc.vector.tensor_copy(ones_r[:], ones_f[:])
            ones_row_f = constp.tile([1, 128], F32)
            nc.vector.memset(ones_row_f[:], 1.0)
            ones_row_r = constp.tile([1, 128], F32R)
            nc.vector.tensor_copy(ones_row_r[:], ones_row_f[:])
            neginv_f = constp.tile([128, 1], F32)
            nc.vector.memset(neginv_f[:], -1.0 / B)
            neginv_r = constp.tile([128, 1], F32R)
            nc.vector.tensor_copy(neginv_r[:], neginv_f[:])
            ones_f8 = constp.tile([128, 1], F8)
            nc.vector.tensor_copy(ones_f8[:], ones_f[:])
            warm = constp.tile([1, 4], F32)
            nc.vector.memset(warm[:], 1.0)
            for fn in (AF.Exp, AF.Sqrt, AF.Ln):
                nc.scalar.activation(warm[:], warm[:], fn)

            # ---- visual input stream: interleaved x/w chunks ----
            xv_sb = inp.tile([128, NKT, 2, S], F8)
            wv_sb = inp.tile([128, NKT, 2, D], F8)
            sizes = [10, 8, 6, 4, 3, 2, 1]
            bounds, k0 = [], 0
            for sz in sizes:
                bounds.append((k0, k0 + sz))
                k0 += sz
            for k0, k1 in bounds:
                nc.sync.dma_start(
                    out=xv_sb[:, k0:k1],
                    in_=xv_d.ap()[k0:k1].rearrange("kt p pl c -> p kt pl c"))
                nc.sync.dma_start(
                    out=wv_sb[:, k0:k1],
                    in_=wv_d.ap()[k0:k1].rearrange("kt p pl c -> p kt pl c"))

            # ---- visual partial E.T, k-outer so PE chases the stream ----
            psum_v = [pbig.tile([128, S], F32, tag=f"pa{d}", name=f"psum_v{d}")
                      for d in range(4)]
            for kt in range(NKT):
                for d in range(4):
                    for h in range(2):
                        nc.tensor.matmul(
                            psum_v[d][:, ts(h, 256)],
                            wv_sb[:, kt, :, ts(d, 128)],
                            xv_sb[:, kt, :, ts(h, 256)],
                            start=(kt == 0 and h == 0), stop=(kt == NKT - 1),
                            perf_mode=DR, skip_group_check=True)

            rs_in = dramp.tile([8, 128, 4, 64], BF16)
            # single (j, d, c)-interleaved staging tile: the payload DMA then
            # has 512-byte contiguous runs on both sides (full DMA rate)
            e_sb = embp.tile([128, 8, 4, 64], BF16, tag="esb", name="e_sb")
            for d in range(4):
                if d % 2 == 0:
                    nc.vector.tensor_copy(
                        e_sb[:, :, d, :],
                        psum_v[d].rearrange("p (j c) -> p j c", j=8))
                else:
                    nc.scalar.copy(
                        e_sb[:, :, d, :],
                        psum_v[d].rearrange("p (j c) -> p j c", j=8))
            nc.sync.dma_start(
                out=rs_in.rearrange("j p d c -> p j d c"), in_=e_sb[:])

            # ---- ReduceScatter: core j gets reduced E.T for samples
            # [64j, 64j+64) as [4, 128, 64] (d-tile, partition, col) ----
            rs_out = dramp.tile([128, 4, 64], BF16)
            nc.gpsimd.collective_compute(
                "ReduceScatter", mybir.AluOpType.add,
                replica_groups=[list(range(N_CORES))],
                ins=[rs_in.opt()], outs=[rs_out.opt()],
            )

            # ---- audio (off the critical path: full K on every core) ----
            # shares the staging slot so the audio input DMA queues AFTER the
            # RS payload write on the DMA engines (WAR on the freed buffer)
            au_sb = embp.tile([128, NKA, 2, 2 * D], F8, tag="esb",
                              name="au_sb")
            nc.sync.dma_start(
                out=au_sb[:],
                in_=au_d.ap().rearrange("kt p pl c -> p kt pl c"))
            psum_a = [pbig.tile([128, S], F32, tag=f"pa{d}", name=f"psum_a{d}")
                      for d in range(4)]
            for d in range(4):
                for kt in range(NKA):
                    for h in range(2):
                        nc.tensor.matmul(
                            psum_a[d][:, ts(h, 256)],
                            au_sb[:, kt, :, D + 128 * d:D + 128 * d + 128],
                            au_sb[:, kt, :, ts(h, 256)],
                            start=(kt == 0 and h == 0), stop=(kt == NKA - 1),
                            perf_mode=DR, skip_group_check=True)
            # audio norms: colsum of squares -> 16/sqrt -> broadcast -> scale
            ax = embp.tile([128, 4, S], F32R)
            for d in range(4):
                nc.vector.tensor_copy(ax[:, d, :], psum_a[d][:])
            sq_a = embp.tile([128, 4, S], F32R)
            for d in range(4):
                nc.vector.tensor_mul(sq_a[:, d, :], ax[:, d, :], ax[:, d, :])
            ps_na = psmall.tile([1, S], F32, tag="rowp", name="ps_na")
            for d in range(4):
                nc.tensor.matmul(ps_na[:], ones_r[:], sq_a[:, d, :],
                                 start=(d == 0), stop=(d == 3))
            sn_a = embp.tile([1, S], F32)
            nc.scalar.activation(sn_a[:], ps_na[:], AF.Sqrt,
                                 scale=float(1.0 / (EMB_SC * EMB_SC)))
            rn_a_f = embp.tile([1, S], F32)
            nc.vector.reciprocal(rn_a_f[:], sn_a[:])
            rn_a = embp.tile([1, S], F32R)
            nc.vector.tensor_copy(rn_a[:], rn_a_f[:])
            ps_bca = psmall.tile([128, S], F32, tag="bcp", name="ps_bca")
            nc.tensor.matmul(ps_bca[:], ones_row_r[:], rn_a[:],
                             start=True, stop=True)
            bc_a = embp.tile([128, S], F32)
            nc.vector.tensor_copy(bc_a[:], ps_bca[:])
            # ea[tt][p, pl, s] = normalized audio emb, d = 128*(2tt+pl)+p
            ea = [embp.tile([128, 2, S], F8, name=f"ea{t}") for t in range(2)]
            for d in range(4):
                nc.vector.tensor_mul(ea[d // 2][:, d % 2, :],
                                     ax[:, d, :], bc_a[:])

            # ---- normalize my reduced 64-sample chunk, AllGather fp8 ----
            red = embp.tile([128, 4, 64], BF16)
            nc.sync.dma_start(out=red[:], in_=rs_out[:])
            sq_v = embp.tile([128, 256], F32R)
            nc.vector.tensor_mul(sq_v[:], red.rearrange("p t c -> p (t c)"),
                                 red.rearrange("p t c -> p (t c)"))
            ps_nv = psmall.tile([1, 64], F32, tag="rowp", name="ps_nv")
            for t in range(4):
                nc.tensor.matmul(ps_nv[:], ones_r[:], sq_v[:, ts(t, 64)],
                                 start=(t == 0), stop=(t == 3))
            sn_v = embp.tile([1, 64], F32)
            nc.scalar.activation(sn_v[:], ps_nv[:], AF.Sqrt,
                                 scale=float(1.0 / (EMB_SC * EMB_SC)))
            rn_v_f = embp.tile([1, 64], F32)
            nc.vector.reciprocal(rn_v_f[:], sn_v[:])
            bc_v = embp.tile([128, 64], F32)
            nc.gpsimd.partition_broadcast(bc_v[:], rn_v_f[:])
            ag_sb = embp.tile([128, 4, 64], F8)
            nc.vector.tensor_mul(ag_sb[:], red[:],
                                 bc_v.rearrange("p (o c) -> p o c", o=1)
                                 .broadcast_to([128, 4, 64]))

            # chunk layout is partition-major so the gathered embeddings can
            # be pulled back in one DMA
            ag_in = dramp.tile([128, 4, 64], F8)
            nc.sync.dma_start(out=ag_in[:], in_=ag_sb[:])
            ag_out = dramp.tile([8, 128, 4, 64], F8)
            nc.gpsimd.collective_compute(
                "AllGather", mybir.AluOpType.bypass,
                replica_groups=[list(range(N_CORES))],
                ins=[ag_in.opt()], outs=[ag_out.opt()],
            )
            # ev[p, j, t, c]: normalized visual emb,
            # d = 128*t+p, sample = 64j+c
            ev = embp.tile([128, 8, 4, 64], F8)
            nc.sync.dma_start(
                out=ev[:], in_=ag_out.rearrange("j p t c -> p j t c"))

            # ---------------- tail: Gram, exp, loss ----------------
            with tc.tile_pool(name="tail", bufs=1) as tp:
                # av Gram: psum_av[m][i, j] = a_{128m+i} . v_j  (x256)
                psum_av = [pbig.tile([128, S], F32, tag=f"pa{m}",
                                     name=f"psum_av{m}") for m in range(4)]
                psum_q = psmall.tile([128, S], F32, tag="bcp", name="psum_q")
                for m in range(4):
                    for tt in range(2):
                        for h in range(2):
                            nc.tensor.matmul(
                                psum_av[m][:, ts(h, 256)],
                                ea[tt][:, :, ts(m, 128)],
                                ev[:, 4 * h:4 * h + 4, 2 * tt:2 * tt + 2]
                                    .rearrange("p j pl c -> p pl j c"),
                                start=(tt == 0 and h == 0), stop=(tt == 1),
                                perf_mode=DR, skip_group_check=True)
                # quadrants: [a1a2 m0, a1a2 m1, v1v2 m0, v1v2 m1]
                for q in range(4):
                    m = q % 2
                    for tt in range(2):
                        if q < 2:    # a1 block m  x  a2 block m
                            lhsT = ea[tt][:, :, ts(m, 128)]
                            rhs = ea[tt][:, :, 256 + 128 * m:384 + 128 * m]
                        else:        # v1 block m  x  v2 block m
                            lhsT = ev[:, 2 * m:2 * m + 2, 2 * tt:2 * tt + 2] \
                                .rearrange("p j pl c -> p pl j c")
                            rhs = ev[:, 4 + 2 * m:6 + 2 * m, 2 * tt:2 * tt + 2] \
                                .rearrange("p j pl c -> p pl j c")
                        nc.tensor.matmul(
                            psum_q[:, ts(q, 128)], lhsT, rhs,
                            start=(q == 0 and tt == 0), stop=(tt == 1),
                            perf_mode=DR, skip_group_check=True)

                # numerator first: raw diagonals straight off the PSUM
                # Grams (no need to wait for the big exps). av block m holds
                # (a?, v1) diag at cols 128j and (a?, v2) at 256+128j, j=m%2.
                mk = tp.tile([128, 4, 256], F32, tag="mk", name="mk")
                qd = tp.tile([128, 12], F32)
                for m in range(4):
                    j = m % 2
                    part = psum_av[m] \
                        .rearrange("p (g c) -> p g c", g=4)[:, j:j + 3:2, :]
                    nc.vector.tensor_mul(
                        mk[:, m].rearrange("p (g c) -> p g c", g=2),
                        part, ident4.rearrange("p (g c) -> p g c",
                                               g=4)[:, 0:2, :])
                    for gi in range(2):
                        col = 6 * (m % 2) + 2 * (m // 2) + gi
                        nc.vector.reduce_sum(
                            qd[:, col:col + 1], mk[:, m, ts(gi, 128)],
                            axis=mybir.AxisListType.X)
                # raw diagonals of (a1,a2) and (v1,v2) quadrants
                mq = tp.tile([128, S], F32, tag="mq", name="mq")
                nc.vector.tensor_mul(mq[:], psum_q[:], ident4[:])
                for q in range(4):
                    col = 6 * (q % 2) + 4 + (q // 2)
                    nc.vector.reduce_sum(qd[:, col:col + 1], mq[:, ts(q, 128)],
                                         axis=mybir.AxisListType.X)

                # denominator: rowsum of exp(G/256) over all 512 visual;
                # the exp'd matrix itself is not needed, only the accum
                junk = tp.tile([128, S], F32, tag="junk", name="junk")
                den4 = tp.tile([128, 4], F32)
                for m in range(4):
                    nc.scalar.activation(junk[:], psum_av[m][:],
                                         AF.Exp, scale=float(GRAM_RCP),
                                         accum_out=den4[:, m:m + 1])
                eqd = tp.tile([128, 12], F32)
                nc.scalar.activation(eqd[:], qd[:], AF.Exp,
                                     scale=float(GRAM_RCP))

                # nd[:, 0:2] = numerator, nd[:, 2:4] = denominator (cols =
                # batch halves)
                nd = tp.tile([128, 4], F32)
                for j in range(2):
                    nc.vector.tensor_add(nd[:, 2 + j:3 + j], den4[:, j:j + 1],
                                         den4[:, j + 2:j + 3])
                # cols [6j, 6j+6) hold all six exp'd numerator terms for
                # batch half j
                for j in range(2):
                    nc.vector.reduce_sum(nd[:, j:j + 1],
                                         eqd[:, 6 * j:6 * j + 6],
                                         axis=mybir.AxisListType.X)

                # loss = -mean(log num - log den); the -1/B mean factor is
                # folded into the summing matmul's stationary vector
                lg = tp.tile([128, 4], F32R)
                nc.scalar.activation(lg[:], nd[:], AF.Ln)
                dif = tp.tile([128, 2], F32R)
                nc.vector.tensor_sub(dif[:], lg[:, 0:2], lg[:, 2:4])
                ps_l = psmall.tile([1, 2], F32, tag="rowp", name="ps_l")
                nc.tensor.matmul(ps_l[:], neginv_r[:], dif[:],
                                 start=True, stop=True)
                loss_sb = tp.tile([1, 1], F32)
                nc.vector.reduce_sum(loss_sb[:], ps_l[:],
                                     axis=mybir.AxisListType.X)
                nc.sync.dma_start(out=loss_d.ap(), in_=loss_sb[:])

    nc.compile()
    return nc


def _get_nc():
    if "nc" not in _CACHE:
        _CACHE["nc"] = build()
    return _CACHE["nc"]


def _pack_kmajor(m, nkt, width):
    """[K, width] k-major -> [nkt, 128, 2, width] fp8 DoubleRow layout."""
    out = m.reshape(nkt, 2, 128, width).transpose(0, 2, 1, 3)
    return np.ascontiguousarray(out).astype(NP_F8)


def _shard_inputs(a_1, v_1, a_2, v_2, W_a, W_v):
    # audio: (2b,1,80,16) -> (512, 1280); replicated on every core
    A = np.concatenate([a_1, a_2], axis=0).reshape(S, KA)
    xa = _pack_kmajor(np.ascontiguousarray(A.T), NKA, S)
    wa = _pack_kmajor(W_a * np.float32(SC_A), NKA, D)
    au = np.ascontiguousarray(np.concatenate([xa, wa], axis=3))
    # visual: lower half, flattened in (c,t,r,w) order; W_v rows permuted
    # from the reference's (t,c) order to match.
    V = np.concatenate([v_1, v_2], axis=0)
    V = V.reshape(S, 15, 96, 96)[:, :, 48:, :].reshape(S, KV_TOT)
    Vt = np.ascontiguousarray(V.T)
    Wvp = np.ascontiguousarray(
        W_v.reshape(5, 3, 48 * 96, D).transpose(1, 0, 2, 3)
    ).reshape(KV_TOT, D) * np.float32(SC_V)

    in_maps = []
    for c in range(N_CORES):
        vt_p = np.zeros((KP, S), np.float32)
        vt_p[:KV] = Vt[c * KV:(c + 1) * KV]
        xv = _pack_kmajor(vt_p, NKT, S)
        wv_p = np.zeros((KP, D), np.float32)
        wv_p[:KV] = Wvp[c * KV:(c + 1) * KV]
        wv = _pack_kmajor(wv_p, NKT, D)
        in_maps.append({"xv": xv, "wv": wv, "au": au})
    return in_maps


def kernel(a_1, v_1, a_2, v_2, W_a, W_v):
    nc = _get_nc()
    in_maps = _shard_inputs(np.asarray(a_1, np.float32),
                            np.asarray(v_1, np.float32),
                            np.asarray(a_2, np.float32),
                            np.asarray(v_2, np.float32),
                            np.asarray(W_a, np.float32),
                            np.asarray(W_v, np.float32))
    res = bass_utils.run_bass_kernel_spmd(nc, in_maps,
                                          core_ids=list(range(N_CORES)))
    return np.asarray(res.results[0]["loss"], np.float32).reshape(())


# revision 51
# speedup vs baseline: 1.0127x; 1.0034x over previous
"""Trainium2 Bass kernel for the audio/visual contrastive loss.

Strategy: K-parallel sharding of the visual matmul in fp8.

- Host casts inputs to fp8-e4m3 (W_v pre-scaled x256, W_a x32 -- any
  per-matrix scale cancels in the L2 normalization) and pre-transposes the
  activations to k-major, so the device does no PE transposes.
- Each core contracts a 8640-wide K slice of the visual matmul with
  fp8 DoubleRow matmuls (two 128-deep k-planes per instruction).
- The audio embedding (K=1280, tiny) is computed fully on every core
  after the visual stream, so the cross-core reduction only carries the
  visual partial E.T and audio stays off the critical path.
- Reduction: bf16 ReduceScatter (each core gets a 64-sample chunk of the
  reduced visual E.T), local L2-normalize of that chunk (scaled x16 for
  fp8 range), then an fp8 AllGather of the normalized embeddings.
- Tail (redundant on every core): fp8 DoubleRow Gram blocks, exp with
  row-accumulate for the denominator, diagonal extraction via
  identity-mask + row-reduce for the numerator, log/mean in column space.
"""

import sys

sys.path.insert(0, "/opt/trn_rl_repo")

import ml_dtypes
import numpy as np

import concourse.bass as bass
import concourse.mybir as mybir
import concourse.tile as tile
from concourse import bacc, bass_utils
from concourse.bass import ts
from concourse.masks import make_identity

N_CORES = 8
B = 256          # batch
S = 2 * B        # samples per modality after the pair-concat
D = 512          # embedding dim
KV_TOT = 3 * 5 * 48 * 96   # 69120 visual features (lower half)
KV = KV_TOT // N_CORES     # 8640 per core
KP = 8704                  # padded to 34 * 256
NKT = KP // 256            # 34 double-k-tiles
KA = 1280                  # audio features (not sharded)
NKA = KA // 256            # 5 double-k-tiles
F32 = mybir.dt.float32
F32R = mybir.dt.float32r
BF16 = mybir.dt.bfloat16
F8 = mybir.dt.float8e4
NP_F8 = ml_dtypes.float8_e4m3
AF = mybir.ActivationFunctionType
DR = mybir.MatmulPerfMode.DoubleRow

SC_V = 256.0    # host scale on W_v so fp8 sees ~unit-variance values
SC_A = 32.0     # host scale on W_a
EMB_SC = 16.0   # scale on normalized embeddings for fp8; Gram gets x256
GRAM_RCP = 1.0 / (EMB_SC * EMB_SC)   # exp(scale * raw_gram)

_CACHE = {}


def build():
    nc = bacc.Bacc("TRN2", target_bir_lowering=False, debug=False,
                   num_devices=N_CORES)

    # k-major fp8 inputs, pre-packed on host for DoubleRow + big DMAs
    xv_d = nc.dram_tensor("xv", [NKT, 128, 2, S], F8, kind="ExternalInput")
    wv_d = nc.dram_tensor("wv", [NKT, 128, 2, D], F8, kind="ExternalInput")
    au_d = nc.dram_tensor("au", [NKA, 128, 2, 2 * D], F8,
                          kind="ExternalInput")
    loss_d = nc.dram_tensor("loss", [1, 1], F32, kind="ExternalOutput")

    with tile.TileContext(nc) as tc:
        with tc.tile_pool(name="const", bufs=1) as constp, \
             tc.tile_pool(name="inp", bufs=1) as inp, \
             tc.tile_pool(name="emb", bufs=1) as embp, \
             tc.tile_pool(name="dram", bufs=1, space="DRAM") as dramp, \
             tc.tile_pool(name="pbig", bufs=1, space="PSUM") as pbig, \
             tc.tile_pool(name="psmall", bufs=1, space="PSUM") as psmall:
            ident = constp.tile([128, 128], F32)
            make_identity(nc, ident[:])
            ident4 = constp.tile([128, S], F32)
            for q in range(4):
                nc.vector.tensor_copy(ident4[:, ts(q, 128)], ident[:])
            ones_f = constp.tile([128, 1], F32)
            nc.vector.memset(ones_f[:], 1.0)
            ones_r = constp.tile([128, 1], F32R)
            nc.vector.tensor_copy(ones_r[:], ones_f[:])
            ones_row_f = constp.tile([1, 128], F32)
            nc.vector.memset(ones_row_f[:], 1.0)
            ones_row_r = constp.tile([1, 128], F32R)
            nc.vector.tensor_copy(ones_row_r[:], ones_row_f[:])
            neginv_f = constp.tile([128, 1], F32)
            nc.vector.memset(neginv_f[:], -1.0 / B)
            neginv_r = constp.tile([128, 1], F32R)
            nc.vector.tensor_copy(neginv_r[:], neginv_f[:])
            ones_f8 = constp.tile([128, 1], F8)
            nc.vector.tensor_copy(ones_f8[:], ones_f[:])
            warm = constp.tile([1, 4], F32)
            nc.vector.memset(warm[:], 1.0)
            for fn in (AF.Exp, AF.Sqrt, AF.Ln):
                nc.scalar.activation(warm[:], warm[:], fn)

            # ---- visual input stream: interleaved x/w chunks ----
            xv_sb = inp.tile([128, NKT, 2, S], F8)
            wv_sb = inp.tile([128, NKT, 2, D], F8)
            sizes = [10, 8, 6, 4, 2, 1, 1, 1, 1]
            bounds, k0 = [], 0
            for sz in sizes:
                bounds.append((k0, k0 + sz))
                k0 += sz
            for k0, k1 in bounds:
                nc.sync.dma_start(
                    out=xv_sb[:, k0:k1],
                    in_=xv_d.ap()[k0:k1].rearrange("kt p pl c -> p kt pl c"))
                nc.sync.dma_start(
                    out=wv_sb[:, k0:k1],
                    in_=wv_d.ap()[k0:k1].rearrange("kt p pl c -> p kt pl c"))

            # ---- visual partial E.T, k-outer so PE chases the stream ----
            psum_v = [pbig.tile([128, S], F32, tag=f"pa{d}", name=f"psum_v{d}")
                      for d in range(4)]
            for kt in range(NKT):
                for d in range(4):
                    for h in range(2):
                        nc.tensor.matmul(
                            psum_v[d][:, ts(h, 256)],
                            wv_sb[:, kt, :, ts(d, 128)],
                            xv_sb[:, kt, :, ts(h, 256)],
                            start=(kt == 0 and h == 0), stop=(kt == NKT - 1),
                            perf_mode=DR, skip_group_check=True)

            rs_in = dramp.tile([8, 128, 4, 64], BF16)
            # single (j, d, c)-interleaved staging tile: the payload DMA then
            # has 512-byte contiguous runs on both sides (full DMA rate)
            e_sb = embp.tile([128, 8, 4, 64], BF16, tag="esb", name="e_sb")
            for d in range(4):
                if d % 2 == 0:
                    nc.vector.tensor_copy(
                        e_sb[:, :, d, :],
                        psum_v[d].rearrange("p (j c) -> p j c", j=8))
                else:
                    nc.scalar.copy(
                        e_sb[:, :, d, :],
                        psum_v[d].rearrange("p (j c) -> p j c", j=8))
            nc.sync.dma_start(
                out=rs_in.rearrange("j p d c -> p j d c"), in_=e_sb[:])

            # ---- ReduceScatter: core j gets reduced E.T for samples
            # [64j, 64j+64) as [4, 128, 64] (d-tile, partition, col) ----
            rs_out = dramp.tile([128, 4, 64], BF16)
            nc.gpsimd.collective_compute(
                "ReduceScatter", mybir.AluOpType.add,
                replica_groups=[list(range(N_CORES))],
                ins=[rs_in.opt()], outs=[rs_out.opt()],
            )

            # ---- audio (off the critical path: full K on every core) ----
            # shares the staging slot so the audio input DMA queues AFTER the
            # RS payload write on the DMA engines (WAR on the freed buffer)
            au_sb = embp.tile([128, NKA, 2, 2 * D], F8, tag="esb",
                              name="au_sb")
            nc.sync.dma_start(
                out=au_sb[:],
                in_=au_d.ap().rearrange("kt p pl c -> p kt pl c"))
            psum_a = [pbig.tile([128, S], F32, tag=f"pa{d}", name=f"psum_a{d}")
                      for d in range(4)]
            for d in range(4):
                for kt in range(NKA):
                    for h in range(2):
                        nc.tensor.matmul(
                            psum_a[d][:, ts(h, 256)],
                            au_sb[:, kt, :, D + 128 * d:D + 128 * d + 128],
                            au_sb[:, kt, :, ts(h, 256)],
                            start=(kt == 0 and h == 0), stop=(kt == NKA - 1),
                            perf_mode=DR, skip_group_check=True)
            # audio norms: colsum of squares -> 16/sqrt -> broadcast -> scale
            ax = embp.tile([128, 4, S], F32R)
            for d in range(4):
                nc.vector.tensor_copy(ax[:, d, :], psum_a[d][:])
            sq_a = embp.tile([128, 4, S], F32R)
            for d in range(4):
                nc.vector.tensor_mul(sq_a[:, d, :], ax[:, d, :], ax[:, d, :])
            ps_na = psmall.tile([1, S], F32, tag="rowp", name="ps_na")
            for d in range(4):
                nc.tensor.matmul(ps_na[:], ones_r[:], sq_a[:, d, :],
                                 start=(d == 0), stop=(d == 3))
            sn_a = embp.tile([1, S], F32)
            nc.scalar.activation(sn_a[:], ps_na[:], AF.Sqrt,
                                 scale=float(1.0 / (EMB_SC * EMB_SC)))
            rn_a_f = embp.tile([1, S], F32)
            nc.vector.reciprocal(rn_a_f[:], sn_a[:])
            rn_a = embp.tile([1, S], F32R)
            nc.vector.tensor_copy(rn_a[:], rn_a_f[:])
            ps_bca = psmall.tile([128, S], F32, tag="bcp", name="ps_bca")
            nc.tensor.matmul(ps_bca[:], ones_row_r[:], rn_a[:],
                             start=True, stop=True)
            bc_a = embp.tile([128, S], F32)
            nc.vector.tensor_copy(bc_a[:], ps_bca[:])
            # ea[tt][p, pl, s] = normalized audio emb, d = 128*(2tt+pl)+p
            ea = [embp.tile([128, 2, S], F8, name=f"ea{t}") for t in range(2)]
            for d in range(4):
                nc.vector.tensor_mul(ea[d // 2][:, d % 2, :],
                                     ax[:, d, :], bc_a[:])

            # ---- normalize my reduced 64-sample chunk, AllGather fp8 ----
            red = embp.tile([128, 4, 64], BF16)
            nc.sync.dma_start(out=red[:], in_=rs_out[:])
            sq_v = embp.tile([128, 256], F32R)
            nc.vector.tensor_mul(sq_v[:], red.rearrange("p t c -> p (t c)"),
                                 red.rearrange("p t c -> p (t c)"))
            ps_nv = psmall.tile([1, 64], F32, tag="rowp", name="ps_nv")
            for t in range(4):
                nc.tensor.matmul(ps_nv[:], ones_r[:], sq_v[:, ts(t, 64)],
                                 start=(t == 0), stop=(t == 3))
            sn_v = embp.tile([1, 64], F32)
            nc.scalar.activation(sn_v[:], ps_nv[:], AF.Sqrt,
                                 scale=float(1.0 / (EMB_SC * EMB_SC)))
            rn_v_f = embp.tile([1, 64], F32)
            nc.vector.reciprocal(rn_v_f[:], sn_v[:])
            bc_v = embp.tile([128, 64], F32)
            nc.gpsimd.partition_broadcast(bc_v[:], rn_v_f[:])
            ag_sb = embp.tile([128, 4, 64], F8)
            nc.vector.tensor_mul(ag_sb[:], red[:],
                                 bc_v.rearrange("p (o c) -> p o c", o=1)
                                 .broadcast_to([128, 4, 64]))

            # chunk layout is partition-major so the gathered embeddings can
            # be pulled back in one DMA
            ag_in = dramp.tile([128, 4, 64], F8)
            nc.sync.dma_start(out=ag_in[:], in_=ag_sb[:])
            ag_out = dramp.tile([8, 128, 4, 64], F8)
            nc.gpsimd.collective_compute(
                "AllGather", mybir.AluOpType.bypass,
                replica_groups=[list(range(N_CORES))],
                ins=[ag_in.opt()], outs=[ag_out.opt()],
            )
            # ev[p, j, t, c]: normalized visual emb,
            # d = 128*t+p, sample = 64j+c
            ev = embp.tile([128, 8, 4, 64], F8)
            nc.sync.dma_start(
                out=ev[:], in_=ag_out.rearrange("j p t c -> p j t c"))

            # ---------------- tail: Gram, exp, loss ----------------
            with tc.tile_pool(name="tail", bufs=1) as tp:
                # av Gram: psum_av[m][i, j] = a_{128m+i} . v_j  (x256)
                psum_av = [pbig.tile([128, S], F32, tag=f"pa{m}",
                                     name=f"psum_av{m}") for m in range(4)]
                psum_q = psmall.tile([128, S], F32, tag="bcp", name="psum_q")
                for m in range(4):
                    for tt in range(2):
                        for h in range(2):
                            nc.tensor.matmul(
                                psum_av[m][:, ts(h, 256)],
                                ea[tt][:, :, ts(m, 128)],
                                ev[:, 4 * h:4 * h + 4, 2 * tt:2 * tt + 2]
                                    .rearrange("p j pl c -> p pl j c"),
                                start=(tt == 0 and h == 0), stop=(tt == 1),
                                perf_mode=DR, skip_group_check=True)
                # quadrants: [a1a2 m0, a1a2 m1, v1v2 m0, v1v2 m1]
                for q in range(4):
                    m = q % 2
                    for tt in range(2):
                        if q < 2:    # a1 block m  x  a2 block m
                            lhsT = ea[tt][:, :, ts(m, 128)]
                            rhs = ea[tt][:, :, 256 + 128 * m:384 + 128 * m]
                        else:        # v1 block m  x  v2 block m
                            lhsT = ev[:, 2 * m:2 * m + 2, 2 * tt:2 * tt + 2] \
                                .rearrange("p j pl c -> p pl j c")
                            rhs = ev[:, 4 + 2 * m:6 + 2 * m, 2 * tt:2 * tt + 2] \
                                .rearrange("p j pl c -> p pl j c")
                        nc.tensor.matmul(
                            psum_q[:, ts(q, 128)], lhsT, rhs,
                            start=(q == 0 and tt == 0), stop=(tt == 1),
                            perf_mode=DR, skip_group_check=True)

                # numerator first: raw diagonals straight off the PSUM
                # Grams (no need to wait for the big exps). av block m holds
                # (a?, v1) diag at cols 128j and (a?, v2) at 256+128j, j=m%2.
                mk = tp.tile([128, 4, 256], F32, tag="mk", name="mk")
                qd = tp.tile([128, 12], F32)
                for m in range(4):
                    j = m % 2
                    part = psum_av[m] \
                        .rearrange("p (g c) -> p g c", g=4)[:, j:j + 3:2, :]
                    nc.vector.tensor_mul(
                        mk[:, m].rearrange("p (g c) -> p g c", g=2),
                        part, ident4.rearrange("p (g c) -> p g c",
                                               g=4)[:, 0:2, :])
                    for gi in range(2):
                        col = 6 * (m % 2) + 2 * (m // 2) + gi
                        nc.vector.reduce_sum(
                            qd[:, col:col + 1], mk[:, m, ts(gi, 128)],
                            axis=mybir.AxisListType.X)
                # raw diagonals of (a1,a2) and (v1,v2) quadrants
                mq = tp.tile([128, S], F32, tag="mq", name="mq")
                nc.vector.tensor_mul(mq[:], psum_q[:], ident4[:])
                for q in range(4):
                    col = 6 * (q % 2) + 4 + (q // 2)
                    nc.vector.reduce_sum(qd[:, col:col + 1], mq[:, ts(q, 128)],
                                         axis=mybir.AxisListType.X)

                # denominator: rowsum of exp(G/256) over all 512 visual;
                # the exp'd matrix itself is not needed, only the accum
                junk = tp.tile([128, S], F32, tag="junk", name="junk")
                den4 = tp.tile([128, 4], F32)
                for m in range(4):
                    nc.scalar.activation(junk[:], psum_av[m][:],
                                         AF.Exp, scale=float(GRAM_RCP),
                                         accum_out=den4[:, m:m + 1])
                eqd = tp.tile([128, 12], F32)
                nc.scalar.activation(eqd[:], qd[:], AF.Exp,
                                     scale=float(GRAM_RCP))

                # nd[:, 0:2] = numerator, nd[:, 2:4] = denominator (cols =
                # batch halves)
                nd = tp.tile([128, 4], F32)
                for j in range(2):
                    nc.vector.tensor_add(nd[:, 2 + j:3 + j], den4[:, j:j + 1],
                                         den4[:, j + 2:j + 3])
                # cols [6j, 6j+6) hold all six exp'd numerator terms for
                # batch half j
                for j in range(2):
                    nc.vector.reduce_sum(nd[:, j:j + 1],
                                         eqd[:, 6 * j:6 * j + 6],
                                         axis=mybir.AxisListType.X)

                # loss = -mean(log num - log den); the -1/B mean factor is
                # folded into the summing matmul's stationary vector
                lg = tp.tile([128, 4], F32R)
                nc.scalar.activation(lg[:], nd[:], AF.Ln)
                dif = tp.tile([128, 2], F32R)
                nc.vector.tensor_sub(dif[:], lg[:, 0:2], lg[:, 2:4])
                ps_l = psmall.tile([1, 2], F32, tag="rowp", name="ps_l")
                nc.tensor.matmul(ps_l[:], neginv_r[:], dif[:],
                                 start=True, stop=True)
                loss_sb = tp.tile([1, 1], F32)
                nc.vector.reduce_sum(loss_sb[:], ps_l[:],
                                     axis=mybir.AxisListType.X)
                nc.sync.dma_start(out=loss_d.ap(), in_=loss_sb[:])

    nc.compile()
    return nc


def _get_nc():
    if "nc" not in _CACHE:
        _CACHE["nc"] = build()
    return _CACHE["nc"]


def _pack_kmajor(m, nkt, width):
    """[K, width] k-major -> [nkt, 128, 2, width] fp8 DoubleRow layout."""
    out = m.reshape(nkt, 2, 128, width).transpose(0, 2, 1, 3)
    return np.ascontiguousarray(out).astype(NP_F8)


def _shard_inputs(a_1, v_1, a_2, v_2, W_a, W_v):
    # audio: (2b,1,80,16) -> (512, 1280); replicated on every core
    A = np.concatenate([a_1, a_2], axis=0).reshape(S, KA)
    xa = _pack_kmajor(np.ascontiguousarray(A.T), NKA, S)
    wa = _pack_kmajor(W_a * np.float32(SC_A), NKA, D)
    au = np.ascontiguousarray(np.concatenate([xa, wa], axis=3))
    # visual: lower half, flattened in (c,t,r,w) order; W_v rows permuted
    # from the reference's (t,c) order to match.
    V = np.concatenate([v_1, v_2], axis=0)
    V = V.reshape(S, 15, 96, 96)[:, :, 48:, :].reshape(S, KV_TOT)
    Vt = np.ascontiguousarray(V.T)
    Wvp = np.ascontiguousarray(
        W_v.reshape(5, 3, 48 * 96, D).transpose(1, 0, 2, 3)
    ).reshape(KV_TOT, D) * np.float32(SC_V)

    in_maps = []
    for c in range(N_CORES):
        vt_p = np.zeros((KP, S), np.float32)
        vt_p[:KV] = Vt[c * KV:(c + 1) * KV]
        xv = _pack_kmajor(vt_p, NKT, S)
        wv_p = np.zeros((KP, D), np.float32)
        wv_p[:KV] = Wvp[c * KV:(c + 1) * KV]
        wv = _pack_kmajor(wv_p, NKT, D)
        in_maps.append({"xv": xv, "wv": wv, "au": au})
    return in_maps


def kernel(a_1, v_1, a_2, v_2, W_a, W_v):
    nc = _get_nc()
    in_maps = _shard_inputs(np.asarray(a_1, np.float32),
                            np.asarray(v_1, np.float32),
                            np.asarray(a_2, np.float32),
                            np.asarray(v_2, np.float32),
                            np.asarray(W_a, np.float32),
                            np.asarray(W_v, np.float32))
    res = bass_utils.run_bass_kernel_spmd(nc, in_maps,
                                          core_ids=list(range(N_CORES)))
    return np.asarray(res.results[0]["loss"], np.float32).reshape(())
